# revision 19
# baseline (speedup 1.0000x reference)
# Differential multi-headed cross-attention on 8 TRN2 NeuronCores (Bass/Tile).
#
# Sharding (hardcoded): core = b * 4 + g  (b in {0,1} batch, g in {0..3} head
# group). Each core handles 4 head-pairs = 8 "half heads" of dim 32, i.e. a
# 256-channel slice of the q/k projections and a 256-channel (4 x 64) slice of
# the v projection, with the full encoder/decoder sequence for its batch.
# Scores never cross cores. The host pre-transposes features/weights so the
# device only does matmuls + exp (contraction on the PE needs e-major layout).
#
# Device-side math per core (channel indices core-local):
#   qT[c, t] = sum_e Wq[c, e] * dec[t, e]      kT[c, n], v[n, c] likewise
#   S_j[n, t] = k_j @ q_j^T                    (scores, transposed layout)
#   E_j = exp(S_j / sqrt(32))                  (ACT engine, PSUM -> SBUF)
#   O_j[r, t] = sum_n vaug_p[n, r] * E_j[n, t] (vaug = [v_p | 1 | -1/lam])
#     rows 0..63 -> unnormalized out^T, row 64 -> denom d_j, row 65 -> -d_j/lam
#   out[t, 64p + r] = O_0[r, t]/d_0[t] + O_1[r, t] * (-lam/d_1[t])
# which equals (softmax(s_2p) - lam * softmax(s_2p+1)) @ v_p of the reference.

import numpy as np

B = 2
T = 2048
N = 2048
E = 1024
LAMBDA_INIT = 0.8
NCORES = 8
GROUPS = 4
PAIRS = 4
CQK = 256
SCALE = 1.0 / np.sqrt(32.0).astype(np.float32)

TCH = 512
EB = E // 128
NTILES = N // 128
NCHUNKS = N // 512
TCHUNKS = T // TCH

_NC_CACHE = {}


def _build_nc(mm_r=True, repeat=1, loop_n=0):
    from contextlib import ExitStack

    import concourse.mybir as mybir
    import concourse.tile as tile
    from concourse import bacc
    from concourse.masks import make_identity

    f32 = mybir.dt.float32
    mm_dt = mybir.dt.float32r if mm_r else mybir.dt.float32
    Exp = mybir.ActivationFunctionType.Exp
    mult = mybir.AluOpType.mult
    add = mybir.AluOpType.add

    nc = bacc.Bacc("TRN2", target_bir_lowering=False, debug=False, num_devices=NCORES)
    encT = nc.dram_tensor("encT", [E, N], mm_dt, kind="ExternalInput")
    decT = nc.dram_tensor("decT", [E, T], mm_dt, kind="ExternalInput")
    wqT = nc.dram_tensor("wqT", [E, CQK], mm_dt, kind="ExternalInput")
    wkT = nc.dram_tensor("wkT", [E, CQK], mm_dt, kind="ExternalInput")
    wvT = nc.dram_tensor("wvT", [E, CQK], mm_dt, kind="ExternalInput")
    aug = nc.dram_tensor("aug", [128, 2], mm_dt, kind="ExternalInput")
    out = nc.dram_tensor("out", [T, CQK], f32, kind="ExternalOutput")

    def bc(ap):
        return ap.bitcast(mm_dt)

    with tile.TileContext(nc) as tc, ExitStack() as ctx:
        pers = ctx.enter_context(tc.tile_pool(name="pers", bufs=1))
        xtp = ctx.enter_context(tc.tile_pool(name="xtp", bufs=2))
        qtp = ctx.enter_context(tc.tile_pool(name="qtp", bufs=2))
        ep = ctx.enter_context(tc.tile_pool(name="ep", bufs=3))
        ocpp = ctx.enter_context(tc.tile_pool(name="ocpp", bufs=2))
        smallp = ctx.enter_context(tc.tile_pool(name="smallp", bufs=4))
        outp = ctx.enter_context(tc.tile_pool(name="outp", bufs=2))
        psum = ctx.enter_context(tc.tile_pool(name="psum", bufs=2, space="PSUM"))

        ident = pers.tile([128, 128], f32)
        make_identity(nc, ident)
        aug_sb = pers.tile([128, 2], mm_dt)
        nc.sync.dma_start(out=aug_sb, in_=aug[:, :])

        wq_sb = pers.tile([128, EB, CQK], mm_dt)
        wk_sb = pers.tile([128, EB, CQK], mm_dt)
        wv_sb = pers.tile([128, EB, CQK], mm_dt)
        for wd, wsb in ((wqT, wq_sb), (wkT, wk_sb), (wvT, wv_sb)):
            nc.sync.dma_start(out=wsb, in_=wd.rearrange("(b p) c -> p b c", p=128))

        kT = [pers.tile([64, N], mm_dt, name=f"kT{p}") for p in range(PAIRS)]
        vaug = [pers.tile([128, PAIRS, 66], mm_dt, name=f"vaug{i}") for i in range(NTILES)]

        def enc_phase():
            # load e-major encoder tiles, project kT (pair layout) and v_aug
            for cn in range(NCHUNKS):
                xt = xtp.tile([128, EB, 512], mm_dt, name="xt", tag="xt")
                nc.sync.dma_start(
                    out=xt,
                    in_=encT[:, cn * 512 : (cn + 1) * 512].rearrange(
                        "(b p) n -> p b n", p=128
                    ),
                )
                for m in range(2):
                    pk = psum.tile([128, 512], f32, name="pk", tag="m")
                    for b in range(EB):
                        nc.tensor.matmul(
                            pk,
                            lhsT=wk_sb[:, b, m * 128 : (m + 1) * 128],
                            rhs=xt[:, b, :],
                            start=(b == 0),
                            stop=(b == EB - 1),
                        )
                    nc.vector.tensor_copy(
                        kT[2 * m][:, cn * 512 : (cn + 1) * 512], bc(pk[0:64, :])
                    )
                    nc.vector.tensor_copy(
                        kT[2 * m + 1][:, cn * 512 : (cn + 1) * 512], bc(pk[64:128, :])
                    )
                for i in range(4):
                    nt = cn * 4 + i
                    pv = psum.tile([128, 256], f32, name="pv", tag="m")
                    for b in range(EB):
                        nc.tensor.matmul(
                            pv,
                            lhsT=xt[:, b, i * 128 : (i + 1) * 128],
                            rhs=wv_sb[:, b, :],
                            start=(b == 0),
                            stop=(b == EB - 1),
                        )
                    nc.vector.tensor_copy(
                        vaug[nt][:, :, 0:64], bc(pv.rearrange("p (h c) -> p h c", h=PAIRS))
                    )
                    for h in range(PAIRS):
                        nc.gpsimd.tensor_copy(vaug[nt][:, h, 64:66], aug_sb)

        def t_chunk(ct):
            xtd = xtp.tile([128, EB, 512], mm_dt, name="xt", tag="xt")
            nc.sync.dma_start(
                out=xtd,
                in_=decT[:, ct * 512 : (ct + 1) * 512].rearrange(
                    "(b p) n -> p b n", p=128
                ),
            )
            qt = [None] * PAIRS
            for m in range(2):
                pq = psum.tile([128, 512], f32, name="pq", tag="m")
                for b in range(EB):
                    nc.tensor.matmul(
                        pq,
                        lhsT=wq_sb[:, b, m * 128 : (m + 1) * 128],
                        rhs=xtd[:, b, :],
                        start=(b == 0),
                        stop=(b == EB - 1),
                    )
                for half in range(2):
                    p = 2 * m + half
                    qm = qtp.tile([64, 512], mm_dt, name=f"qt{p}", tag=f"qt{p}")
                    nc.vector.tensor_copy(qm, bc(pq[64 * half : 64 * half + 64, :]))
                    qt[p] = qm

            otile = outp.tile([128, 4, CQK], f32, name="otile", tag="otile")
            for p in range(PAIRS):
                op = psum.tile([66, 1024], f32, name="op", tag="o", bufs=1)
                for nt in range(NTILES):
                    sp = psum.tile([128, 1024], f32, name="sp", tag="s")
                    nc.tensor.matmul(
                        sp[:, 0:512],
                        lhsT=kT[p][0:32, nt * 128 : (nt + 1) * 128],
                        rhs=qt[p][0:32, :],
                        start=True,
                        stop=True,
                    )
                    nc.tensor.matmul(
                        sp[:, 512:1024],
                        lhsT=kT[p][32:64, nt * 128 : (nt + 1) * 128],
                        rhs=qt[p][32:64, :],
                        start=True,
                        stop=True,
                    )
                    et = ep.tile([128, 1024], mm_dt, name="et", tag="et")
                    nc.scalar.activation(et, sp, Exp, scale=float(SCALE))
                    nc.tensor.matmul(
                        op[:, 0:512],
                        lhsT=vaug[nt][:, p, :],
                        rhs=et[:, 0:512],
                        start=(nt == 0),
                        stop=(nt == NTILES - 1),
                        skip_group_check=True,
                    )
                    nc.tensor.matmul(
                        op[:, 512:1024],
                        lhsT=vaug[nt][:, p, :],
                        rhs=et[:, 512:1024],
                        start=(nt == 0),
                        stop=(nt == NTILES - 1),
                        skip_group_check=True,
                    )
                ocp = ocpp.tile([66, 1024], f32, name="ocp", tag="ocp")
                nc.vector.tensor_copy(ocp, op)
                ots = []
                for h in range(2):
                    ot = psum.tile([128, 4, 66], f32, name=f"ot{h}", tag="m")
                    for s in range(4):
                        nc.tensor.transpose(
                            ot[:, s, :],
                            ocp[:, h * 512 + s * 128 : h * 512 + (s + 1) * 128],
                            ident[0:66, 0:66],
                        )
                    ots.append(ot)
                rr0 = smallp.tile([128, 4], f32, name="rr0", tag="rr")
                nc.vector.reciprocal(rr0, ots[0][:, :, 64])
                rr1 = smallp.tile([128, 4], f32, name="rr1", tag="rr")
                nc.vector.reciprocal(rr1, ots[1][:, :, 65])
                for s in range(4):
                    tmp = smallp.tile([128, 64], f32, name="tmp", tag="tmp")
                    nc.vector.tensor_scalar_mul(
                        tmp, ots[1][:, s, 0:64], rr1[:, s : s + 1]
                    )
                    nc.vector.scalar_tensor_tensor(
                        out=otile[:, s, p * 64 : (p + 1) * 64],
                        in0=ots[0][:, s, 0:64],
                        scalar=rr0[:, s : s + 1],
                        in1=tmp,
                        op0=mult,
                        op1=add,
                    )
            nc.sync.dma_start(
                out=out[ct * TCH : (ct + 1) * TCH, :].rearrange(
                    "(s p) c -> p s c", p=128
                ),
                in_=otile,
            )

        def body():
            enc_phase()
            for ct in range(TCHUNKS):
                t_chunk(ct)

        if loop_n:
            # device-side repeat loop (timing variant); body is idempotent
            with tc.For_i(0, loop_n, 1):
                body()
        else:
            for _rep in range(repeat):
                body()
    nc.finalize()
    return nc


def _get_nc_loop(loop_n, mm_r=True):
    key = ("loop", mm_r, loop_n)
    if key not in _NC_CACHE:
        _NC_CACHE[key] = _build_nc(mm_r, loop_n=loop_n)
    return _NC_CACHE[key]


def _get_nc(mm_r=True, repeat=1):
    key = (mm_r, repeat)
    if key not in _NC_CACHE:
        _NC_CACHE[key] = _build_nc(mm_r, repeat)
    return _NC_CACHE[key]


def _make_in_maps(
    encoder_feature, decoder_feature, Wq, Wk, Wv, lambda_q1, lambda_q2, lambda_k1, lambda_k2
):
    lam = float(
        np.exp(np.sum(np.float32(lambda_q1) * np.float32(lambda_k1), dtype=np.float32))
        - np.exp(np.sum(np.float32(lambda_q2) * np.float32(lambda_k2), dtype=np.float32))
        + np.float32(LAMBDA_INIT)
    )
    aug = np.empty((128, 2), np.float32)
    aug[:, 0] = 1.0
    aug[:, 1] = -1.0 / lam
    encTs = [
        np.ascontiguousarray(np.asarray(encoder_feature[b], dtype=np.float32).T)
        for b in range(B)
    ]
    decTs = [
        np.ascontiguousarray(np.asarray(decoder_feature[b], dtype=np.float32).T)
        for b in range(B)
    ]
    WqT = np.asarray(Wq, dtype=np.float32).T
    WkT = np.asarray(Wk, dtype=np.float32).T
    WvT = np.asarray(Wv, dtype=np.float32).T
    in_maps = []
    for core in range(NCORES):
        b, g = divmod(core, GROUPS)
        c0 = g * CQK
        in_maps.append(
            {
                "encT": encTs[b],
                "decT": decTs[b],
                "wqT": np.ascontiguousarray(WqT[:, c0 : c0 + CQK]),
                "wkT": np.ascontiguousarray(WkT[:, c0 : c0 + CQK]),
                "wvT": np.ascontiguousarray(WvT[:, c0 : c0 + CQK]),
                "aug": aug,
            }
        )
    return in_maps


def kernel(
    encoder_feature,
    decoder_feature,
    Wq,
    Wk,
    Wv,
    lambda_q1,
    lambda_q2,
    lambda_k1,
    lambda_k2,
    _repeat=1,
):
    from concourse.bass_utils import run_bass_kernel_spmd

    nc = _get_nc(repeat=_repeat)
    in_maps = _make_in_maps(
        encoder_feature, decoder_feature, Wq, Wk, Wv,
        lambda_q1, lambda_q2, lambda_k1, lambda_k2,
    )
    res = run_bass_kernel_spmd(nc, in_maps, list(range(NCORES)))
    full = np.empty((B, T, E), np.float32)
    for core in range(NCORES):
        b, g = divmod(core, GROUPS)
        full[b, :, g * CQK : (g + 1) * CQK] = res.results[core]["out"]
    return full
